# revision 1
# baseline (speedup 1.0000x reference)
"""Trainium2 Bass kernel for multi-head attention (dense_transformer).

Full module: qkv = x @ W_qkv + b_qkv; multi-head attention (16 heads, d=64,
N=4096); out = attn @ W_proj + b_proj.

Sharding: tensor-parallel over heads — 2 heads per core on 8 cores. Each core
receives full x (pre-transposed on host to [C, N]) plus its head-slices of the
weights, computes its heads' attention and a partial output projection
[N, C]; the host sums the 8 partials and adds b_proj.

Per-core dataflow (matmul operands in fp16 — 16-bit streaming is ~5x
faster than fp32/fp32r on the PE moving-operand path; PSUM accumulates fp32):
  A) Q^T,K^T [128, 4096] = W^T @ x^T accumulated over C chunks (PSUM), bias
     added on eviction.  V^T likewise, then PE-transposed to V natural
     [tok, d] stored with a constant ones column per head: [V_h | 1].
  B) per (q-chunk 512, k-chunk 128): S^T [128, 1024] for both heads packed
     side by side (row-tiled K=64 matmuls), ACT exp (scale=1/8) from PSUM to
     SBUF fp16, then AV matmuls lhsT=[V_h|1] accumulate out^T[65, 512] whose
     row 64 is the softmax denominator.  Normalize with DVE reciprocal +
     GPSIMD partition broadcast + DVE mul, add b_v.
  C) partial proj: out[tok,  C] = attn_out^T.T @ W_proj_slice, evicted by DVE
     and DMAd out.
"""

import numpy as np
from contextlib import ExitStack

NUM_CORES = 8
DIM = 1024
NUM_HEADS = 16
HDIM = 64
N = 4096
HPC = NUM_HEADS // NUM_CORES   # heads per core = 2
DPC = HPC * HDIM               # head dims per core = 128

_NC_CACHE = {}


def build_nc(reps=1, trace_sim=False):
    if (reps, trace_sim) in _NC_CACHE:
        return _NC_CACHE[(reps, trace_sim)]

    import concourse.bass as bass
    import concourse.mybir as mybir
    import concourse.tile as tile
    from concourse import bacc
    from concourse.masks import make_identity

    f32 = mybir.dt.float32
    fp16 = mybir.dt.float16
    AF = mybir.ActivationFunctionType
    ts = bass.ts

    nc = bacc.Bacc(trn_type="TRN2", target_bir_lowering=False, debug=False)
    xT = nc.dram_tensor("xT", [DIM, N], fp16, kind="ExternalInput").ap()
    wq = nc.dram_tensor("wq", [DIM, DPC], fp16, kind="ExternalInput").ap()
    wk = nc.dram_tensor("wk", [DIM, DPC], fp16, kind="ExternalInput").ap()
    wv = nc.dram_tensor("wv", [DIM, DPC], fp16, kind="ExternalInput").ap()
    wp = nc.dram_tensor("wp", [DPC, DIM], fp16, kind="ExternalInput").ap()
    bq = nc.dram_tensor("bq", [DPC, 1], f32, kind="ExternalInput").ap()
    bk = nc.dram_tensor("bk", [DPC, 1], f32, kind="ExternalInput").ap()
    bv = nc.dram_tensor("bv", [DPC, 1], f32, kind="ExternalInput").ap()
    ones = nc.dram_tensor("ones", [1, 1], fp16, kind="ExternalInput").ap()
    out = nc.dram_tensor("out", [N, DIM], f32, kind="ExternalOutput").ap()

    with tile.TileContext(nc, trace_sim=trace_sim) as tc, ExitStack() as ctx:
        singles = ctx.enter_context(tc.tile_pool(name="singles", bufs=1))
        psum = ctx.enter_context(tc.tile_pool(name="ps", bufs=2, space="PSUM"))
        xpool = ctx.enter_context(tc.tile_pool(name="xp", bufs=2))
        work = ctx.enter_context(tc.tile_pool(name="work", bufs=2))
        ppool = ctx.enter_context(tc.tile_pool(name="pp", bufs=3))
        opool = ctx.enter_context(tc.tile_pool(name="op", bufs=3))

        ident = singles.tile([128, 128], f32, tag="ident")
        make_identity(nc, ident)

        wq_sb = singles.tile([128, 8, DPC], fp16, tag="wq")
        wk_sb = singles.tile([128, 8, DPC], fp16, tag="wk")
        wv_sb = singles.tile([128, 8, DPC], fp16, tag="wv")
        nc.sync.dma_start(out=wq_sb, in_=wq.rearrange("(c p) m -> p c m", p=128))
        nc.sync.dma_start(out=wk_sb, in_=wk.rearrange("(c p) m -> p c m", p=128))
        nc.sync.dma_start(out=wv_sb, in_=wv.rearrange("(c p) m -> p c m", p=128))
        wp_sb = singles.tile([64, HPC, DIM], fp16, tag="wp")
        nc.sync.dma_start(out=wp_sb, in_=wp.rearrange("(h d) c -> d h c", d=64))
        bq_sb = singles.tile([DPC, 1], f32, tag="bq")
        bk_sb = singles.tile([DPC, 1], f32, tag="bk")
        nc.sync.dma_start(out=bq_sb, in_=bq)
        nc.sync.dma_start(out=bk_sb, in_=bk)
        bv_sb = singles.tile([64, HPC, 1], f32, tag="bv")
        nc.sync.dma_start(out=bv_sb, in_=bv.rearrange("(h d) x -> d h x", d=64))

        qT = singles.tile([128, N], fp16, tag="qT")
        kT = singles.tile([128, N], fp16, tag="kT")
        aoT0 = singles.tile([64, N], fp16, tag="aoT0")
        aoT1 = singles.tile([64, N], fp16, tag="aoT1")
        # V natural layout + ones column per head: [.., t, 0:64]=V_h0,
        # [.., t, 64]=1, [.., t, 65:129]=V_h1, [.., t, 129]=1
        v_nat = singles.tile([128, 32, 130], fp16, tag="vnat")
        # ones columns loaded via broadcast DMA
        nc.sync.dma_start(out=v_nat[:, :, 64:65], in_=ones.to_broadcast((128, 32, 1)))
        nc.sync.dma_start(out=v_nat[:, :, 129:130], in_=ones.to_broadcast((128, 32, 1)))

        for _rep in range(reps):
            # ---------------- Phase A: QKV projection ----------------
            for qt in range(4):
                xt = [xpool.tile([128, 1024], fp16, tag=f"x{c}", name=f"x{c}") for c in range(8)]
                for c in range(8):
                    nc.sync.dma_start(out=xt[c], in_=xT[ts(c, 128), ts(qt, 1024)])
                for nl in range(2):
                    n = qt * 2 + nl
                    # K and V first: attention waits on full K/V, while Q
                    # chunks are consumed per q-tile
                    acc = psum.tile([128, 512], f32, tag="pj", name="kacc", bufs=2)
                    for c in range(8):
                        nc.tensor.matmul(
                            acc, wk_sb[:, c, :], xt[c][:, ts(nl, 512)],
                            start=(c == 0), stop=(c == 7),
                        )
                    nc.vector.tensor_scalar_add(kT[:, ts(n, 512)], acc, bk_sb)
                    vacc = psum.tile([128, 512], f32, tag="pj", name="vacc", bufs=2)
                    for c in range(8):
                        nc.tensor.matmul(
                            vacc, wv_sb[:, c, :], xt[c][:, ts(nl, 512)],
                            start=(c == 0), stop=(c == 7),
                        )
                    vst = work.tile([128, 512], f32, tag="vst")
                    nc.vector.tensor_copy(vst, vacc)
                    tpb = psum.tile([128, 512], f32, tag="big", name="tpb", bufs=2)
                    for tl in range(4):
                        nc.tensor.transpose(
                            tpb[:, ts(tl, 128)], vst[:, ts(tl, 128)], ident)
                    nc.vector.tensor_copy(
                        out=v_nat[:, ts(n, 4), 0:130]
                        .rearrange("p t (g d) -> p t g d", d=65)[:, :, :, 0:64],
                        in_=tpb.rearrange("p (t g d) -> p t g d", g=2, d=64),
                    )
                    qacc = psum.tile([128, 512], f32, tag="big", name="qacc", bufs=2)
                    for c in range(8):
                        nc.tensor.matmul(
                            qacc, wq_sb[:, c, :], xt[c][:, ts(nl, 512)],
                            start=(c == 0), stop=(c == 7),
                        )
                    nc.vector.tensor_scalar_add(qT[:, ts(n, 512)], qacc, bq_sb)

            # ---------------- Phase B: attention + lagged projection ------
            def emit_proj_chunk(t, j):
                pp = psum.tile([128, 512], f32, tag="pj", name="pp", bufs=2)
                nc.tensor.matmul(
                    pp, aoT0[:, ts(t, 128)], wp_sb[:, 0, ts(j, 512)],
                    start=True, stop=False,
                )
                nc.tensor.matmul(
                    pp, aoT1[:, ts(t, 128)], wp_sb[:, 1, ts(j, 512)],
                    start=False, stop=True,
                )
                ot = opool.tile([128, 512], f32, tag="ot")
                nc.vector.tensor_copy(ot, pp)
                nc.sync.dma_start(out=out[ts(t, 128), ts(j, 512)], in_=ot)

            for qi in range(8):
                # proj tasks for the previous q-chunk, spread across this
                # q-chunk's ki loop so they fill PE slack without stalling
                # the score/exp stream
                proj_tasks = (
                    [((qi - 1) * 4 + tl, j) for tl in range(4) for j in range(2)]
                    if qi >= 1 else []
                )
                av = [
                    psum.tile([65, 512], f32, tag="av0", name="av0", bufs=1),
                    psum.tile([65, 512], f32, tag="av1", name="av1", bufs=1),
                ]
                # software-pipelined: emit scores/exp one step ahead of AV
                p_tiles = {}
                for ki in range(33):
                    if ki < 32:
                        s = psum.tile([128, 1024], f32, tag="big", name="s", bufs=2)
                        nc.tensor.matmul(
                            s[:, 0:512], kT[0:64, ts(ki, 128)], qT[0:64, ts(qi, 512)],
                            start=True, stop=True,
                        )
                        nc.tensor.matmul(
                            s[:, 512:1024], kT[64:128, ts(ki, 128)],
                            qT[64:128, ts(qi, 512)],
                            start=True, stop=True,
                        )
                        p = ppool.tile([128, 1024], fp16, tag="p")
                        nc.scalar.activation(p, s, AF.Exp, scale=0.125)
                        p_tiles[ki] = p
                    if ki >= 1:
                        kj = ki - 1
                        p = p_tiles.pop(kj)
                        nc.tensor.matmul(
                            av[0], v_nat[:, kj, 0:65], p[:, 0:512],
                            start=(kj == 0), stop=(kj == 31),
                        )
                        nc.tensor.matmul(
                            av[1], v_nat[:, kj, 65:130], p[:, 512:1024],
                            start=(kj == 0), stop=(kj == 31),
                        )
                    if ki % 4 == 2 and proj_tasks:
                        emit_proj_chunk(*proj_tasks.pop(0))
                for h, (acc, aoT) in enumerate(((av[0], aoT0), (av[1], aoT1))):
                    # single fast copy releases the PSUM accumulator slot so
                    # the next q-chunk's AV matmuls aren't gated on the whole
                    # normalize chain; normalize runs from the SBUF staging
                    avs = work.tile([65, 512], f32, tag="avs", name="avs",
                                    bufs=4)
                    nc.vector.tensor_copy(avs, acc)
                    recip = work.tile([1, 512], f32, tag="recip", name="recip")
                    nc.vector.reciprocal(recip, avs[64:65, :])
                    bc = work.tile([64, 512], f32, tag="bc", name="bc")
                    nc.gpsimd.partition_broadcast(bc, recip)
                    nc.vector.tensor_mul(aoT[:, ts(qi, 512)], avs[0:64, :], bc)
                    nc.vector.tensor_scalar_add(
                        aoT[:, ts(qi, 512)], aoT[:, ts(qi, 512)], bv_sb[:, h, :]
                    )
            # tail: projection of the final q-chunk
            for tl in range(4):
                for j in range(2):
                    emit_proj_chunk(7 * 4 + tl, j)

    nc.compile()
    _NC_CACHE[(reps, trace_sim)] = nc
    return nc


def make_in_maps(x, W_qkv, b_qkv, W_proj):
    x2 = np.asarray(x, dtype=np.float32).reshape(N, DIM)
    xTv = np.ascontiguousarray(x2.T.astype(np.float16))
    W_qkv = np.asarray(W_qkv, dtype=np.float32)
    W16 = W_qkv.astype(np.float16)
    b_qkv = np.asarray(b_qkv, dtype=np.float32)
    Wp16 = np.asarray(W_proj, dtype=np.float32).astype(np.float16)
    maps = []
    for m in range(NUM_CORES):
        h0 = m * DPC
        maps.append({
            "xT": xTv,
            "wq": np.ascontiguousarray(W16[:, h0:h0 + DPC]),
            "wk": np.ascontiguousarray(W16[:, DIM + h0:DIM + h0 + DPC]),
            "wv": np.ascontiguousarray(W16[:, 2 * DIM + h0:2 * DIM + h0 + DPC]),
            "wp": np.ascontiguousarray(Wp16[h0:h0 + DPC, :]),
            "bq": np.ascontiguousarray(b_qkv[h0:h0 + DPC].reshape(DPC, 1)),
            "bk": np.ascontiguousarray(
                b_qkv[DIM + h0:DIM + h0 + DPC].reshape(DPC, 1)),
            "bv": np.ascontiguousarray(
                b_qkv[2 * DIM + h0:2 * DIM + h0 + DPC].reshape(DPC, 1)),
            "ones": np.ones((1, 1), dtype=np.float16),
        })
    return maps


def kernel(x, W_qkv, b_qkv, W_proj, b_proj, _reps=1):
    from concourse.bass_utils import run_bass_kernel_spmd

    nc = build_nc(_reps)
    maps = make_in_maps(x, W_qkv, b_qkv, W_proj)
    res = run_bass_kernel_spmd(nc, maps, list(range(NUM_CORES)))
    partial = np.stack([r["out"] for r in res.results], axis=0)
    total = partial.sum(axis=0, dtype=np.float32)
    total = total + np.asarray(b_proj, dtype=np.float32)[None, :]
    return total.reshape(1, N, DIM).astype(np.float32)



# revision 54
# speedup vs baseline: 3.5490x; 3.5490x over previous
"""Trainium2 Bass kernel for multi-head attention (dense_transformer).

Full module: qkv = x @ W_qkv + b_qkv; 16-head attention (d=64, N=4096);
out = attn @ W_proj + b_proj.

Sharding: tensor-parallel over heads - 2 heads per core on 8 cores. Each core
gets full x (pre-transposed to [C, N] fp16) plus its head slices of the
weights, computes its heads' attention and a partial projection [N, C] fp16;
the host sums the 8 partials in fp32 and adds b_proj.

Per-core dataflow (all matmul operands fp16; PSUM accumulates fp32):
  Prefix: x^T resident in SBUF; K^T [128, N] = Wk^T @ x^T (Wk stationary),
     bias on DVE eviction.  V computed directly in natural [tok, d] layout
     (x^T tiles stationary, Wv moving) - first 4 key-blocks in the prefix,
     the rest just-in-time inside attention chunk 0 - stored per key-block
     as [V_h | 1] per head; the V bias is added on eviction (equivalent to
     biasing the attention output since softmax weights are normalized).
     Q^T chunk 0 last; later Q chunks are produced just-in-time at the end
     of the previous attention chunk, overlapping the normalize.
  Attention, per (q-chunk 256, key-block 128), software-pipelined 3 deep:
     row-tiled concurrent score matmuls (both heads, K=64 in disjoint PE
     row groups) into separate PSUM banks of a [128,1024] tile (two
     matmuls into ONE bank fault the device - see below); exp from PSUM
     split between ACT (exact, scale=1/8, ~19/32 of key blocks) and DVE
     (Schraudolph: fp16 bits = s*SCH_A + SCH_B via one fused mul-add into
     a uint16 view of the p tile).  AV uses p-slices [key, 128q] as the
     stationary operand and [V_h | 1] as moving, accumulating av[q, 65]
     per (head, q-tile) group - column 64 is the softmax denominator.  All
     4 groups share one PSUM bank at 68-elem stride: only the first group
     may use start=True since start clears has_written bank-wide; later
     groups overwrite-where-unset.  (Two start=True matmuls - or even
     start=False ones - aimed at one bank in back-to-back *separate*
     score tiles hard-fault the device, hence the two-bank score layout.)
  Deferred per-chunk epilogue (runs inside the NEXT chunk's loop to keep
     PE dense): per q-tile, per-partition reciprocal + tensor_scalar
     normalize into ao_nat fp16, PE transpose packs both heads into aoT
     [d=128, tok], then one K=128 proj matmul per (tok-tile, col-half)
     with aoT stationary, evicted fp16 (ACT/DVE alternating) and DMAd out.
"""

import numpy as np
from contextlib import ExitStack

NUM_CORES = 8
DIM = 1024
NUM_HEADS = 16
HDIM = 64
N = 4096
HPC = NUM_HEADS // NUM_CORES   # heads per core = 2
DPC = HPC * HDIM               # head dims per core = 128

# Schraudolph exp: fp16 bits = s*SCH_A + SCH_B  (s = raw score, logit = s/8)
SCH_A = 0.125 * 1024.0 * 1.4426950408889634
SCH_B = 15360.0 - 44.0
# key-blocks whose exp runs on ACT (19/32); DVE takes odd blocks 3..27 so it
# is free for the chunk-boundary normalize
ACT_KIS = {0, 1, 2, 31} | set(range(4, 32, 2))

_NC_CACHE = {}


def build_nc(reps=1, trace_sim=False):
    if (reps, trace_sim) in _NC_CACHE:
        return _NC_CACHE[(reps, trace_sim)]

    import concourse.bass as bass
    import concourse.mybir as mybir
    import concourse.tile as tile
    from concourse import bacc
    from concourse.masks import make_identity

    f32 = mybir.dt.float32
    fp16 = mybir.dt.float16
    u16 = mybir.dt.uint16
    AF = mybir.ActivationFunctionType
    ALU = mybir.AluOpType
    ts = bass.ts

    nc = bacc.Bacc(trn_type="TRN2", target_bir_lowering=False, debug=False)
    xT = nc.dram_tensor("xT", [DIM, N], fp16, kind="ExternalInput").ap()
    wq = nc.dram_tensor("wq", [DIM, DPC], fp16, kind="ExternalInput").ap()
    wk = nc.dram_tensor("wk", [DIM, DPC], fp16, kind="ExternalInput").ap()
    wv = nc.dram_tensor("wv", [DIM, DPC], fp16, kind="ExternalInput").ap()
    wp = nc.dram_tensor("wp", [DPC, DIM], fp16, kind="ExternalInput").ap()
    bq = nc.dram_tensor("bq", [DPC, 1], f32, kind="ExternalInput").ap()
    bk = nc.dram_tensor("bk", [DPC, 1], f32, kind="ExternalInput").ap()
    bv = nc.dram_tensor("bv", [1, DPC], fp16, kind="ExternalInput").ap()
    ones = nc.dram_tensor("ones", [1, 1], fp16, kind="ExternalInput").ap()
    out = nc.dram_tensor("out", [N, DIM], fp16, kind="ExternalOutput").ap()

    with tile.TileContext(nc, trace_sim=trace_sim) as tc, ExitStack() as ctx:
        singles = ctx.enter_context(tc.tile_pool(name="singles", bufs=1))
        psum = ctx.enter_context(tc.tile_pool(name="ps", bufs=2, space="PSUM"))
        work = ctx.enter_context(tc.tile_pool(name="work", bufs=2))
        ppool = ctx.enter_context(tc.tile_pool(name="pp", bufs=5))
        opool = ctx.enter_context(tc.tile_pool(name="op", bufs=3))

        ident = singles.tile([128, 128], fp16, tag="ident")
        make_identity(nc, ident)
        # preload the exp table set so the first real exp pays no ~2.7us load
        warm = singles.tile([1, 1], f32, tag="warm")
        nc.gpsimd.memset(warm, 0.0)
        nc.scalar.activation(warm, warm, mybir.ActivationFunctionType.Exp)

        wq_sb = singles.tile([128, 8, DPC], fp16, tag="wq")
        wk_sb = singles.tile([128, 8, DPC], fp16, tag="wk")
        wv_sb = singles.tile([128, 8, DPC], fp16, tag="wv")
        nc.sync.dma_start(out=wq_sb, in_=wq.rearrange("(c p) m -> p c m", p=128))
        nc.sync.dma_start(out=wk_sb, in_=wk.rearrange("(c p) m -> p c m", p=128))
        nc.sync.dma_start(out=wv_sb, in_=wv.rearrange("(c p) m -> p c m", p=128))
        wp_sb = singles.tile([DPC, DIM], fp16, tag="wp")
        nc.sync.dma_start(out=wp_sb, in_=wp)
        bq_sb = singles.tile([DPC, 1], f32, tag="bq")
        bk_sb = singles.tile([DPC, 1], f32, tag="bk")
        nc.sync.dma_start(out=bq_sb, in_=bq)
        nc.sync.dma_start(out=bk_sb, in_=bk)
        # V bias broadcast across partitions: [128, (h d)]
        bv_sb = singles.tile([128, HPC, HDIM], fp16, tag="bv")
        nc.sync.dma_start(
            out=bv_sb,
            in_=bv.rearrange("x (h d) -> x h d", h=HPC).to_broadcast(
                (128, HPC, HDIM)
            ),
        )

        # full x^T resident: [c-part, c-chunk, tok]
        xf = singles.tile([128, 8, N], fp16, tag="xf")
        qT = singles.tile([128, N], fp16, tag="qT")
        kT = singles.tile([128, N], fp16, tag="kT")
        aoT = singles.tile([128, N], fp16, tag="aoT")
        # V natural layout: [key-part, key-block*head, 65]; col 64 = ones
        v_nat = singles.tile([128, 32 * HPC, 65], fp16, tag="vnat")
        nc.sync.dma_start(
            out=v_nat[:, :, 64:65], in_=ones.to_broadcast((128, 32 * HPC, 1))
        )

        # q-chunks of 256: scores tile = 1 bank (bufs=4), av packs all 4
        # (head, q-tile) groups of 65 at stride 68 in ONE bank (bufs=2),
        # proj pool keeps 2 bufs: 4+2+2 = 8 banks
        AVS = 68

        def emit_q_chunk(qi):
            acc = psum.tile([128, 256], f32, tag="pj", name="qacc", bufs=1)
            for c in range(8):
                nc.tensor.matmul(
                    acc, wq_sb[:, c, :], xf[:, c, ts(qi, 256)],
                    start=(c == 0), stop=(c == 7),
                )
            nc.vector.tensor_scalar_add(qT[:, ts(qi, 256)], acc, bq_sb)

        def emit_proj(t, j, evict="act"):
            pp = psum.tile([128, 512], f32, tag="pj", name="pp", bufs=1)
            nc.tensor.matmul(
                pp, aoT[:, ts(t, 128)], wp_sb[:, ts(j, 512)],
                start=True, stop=True,
            )
            ot = opool.tile([128, 512], fp16, tag="ot")
            # alternate eviction engine: ACT is near-saturated mid-chunk
            if evict == "act":
                nc.scalar.activation(ot, pp, AF.Copy)
            else:
                nc.vector.tensor_copy(ot, pp)
            nc.sync.dma_start(out=out[ts(t, 128), ts(j, 512)], in_=ot)

        def emit_normalize(qi, av):
            # per (q-tile, head) group: 1/denominator then scale into ao_nat
            # fp16; group g = h*2+qt at column offset g*AVS, col 64 = denom
            aos = [
                work.tile([128, 128], fp16, tag="ao", name="ao", bufs=5)
                for _ in range(2)
            ]
            for h in range(2):
                for qt in range(2):
                    g = h * 2 + qt
                    recip = work.tile([128, 1], f32, tag="rc", name="rc",
                                      bufs=4)
                    nc.vector.reciprocal(
                        recip, av[:, g * AVS + 64:g * AVS + 65])
                    nc.vector.tensor_scalar(
                        aos[qt][:, ts(h, 64)],
                        av[:, g * AVS:g * AVS + 64],
                        recip, None, ALU.mult,
                    )
            return aos

        def emit_transpose(qi, qt, ao_nat):
            tp = psum.tile([128, 128], fp16, tag="pj", name="tp", bufs=1)
            nc.tensor.transpose(tp, ao_nat, ident)
            # ACT eviction: keeps the aoT copy off the DVE exp stream
            nc.scalar.copy(aoT[:, ts(qi * 2 + qt, 128)], tp)

        for _rep in range(reps):
            # ---------------- prefix: x load, K all, V all, Q chunk 0 -------
            for n in range(8):
                for c in range(8):
                    nc.sync.dma_start(
                        out=xf[:, c, ts(n, 512)], in_=xT[ts(c, 128), ts(n, 512)]
                    )
            def emit_v(kj):
                vacc = psum.tile([128, 512], f32, tag="pj", name="vacc",
                                 bufs=1)[:, 0:128]
                for c in range(8):
                    nc.tensor.matmul(
                        vacc, xf[:, c, ts(kj, 128)], wv_sb[:, c, :],
                        start=(c == 0), stop=(c == 7),
                    )
                nc.vector.tensor_tensor(
                    out=v_nat[:, 2 * kj:2 * kj + 2, 0:64],
                    in0=vacc.rearrange("p (h d) -> p h d", h=HPC),
                    in1=bv_sb,
                    op=ALU.add,
                )

            for n in range(8):
                kacc = psum.tile([128, 1024], f32, tag="big", name="kacc",
                                 bufs=3)[:, 0:512]
                for c in range(8):
                    nc.tensor.matmul(
                        kacc, wk_sb[:, c, :], xf[:, c, ts(n, 512)],
                        start=(c == 0), stop=(c == 7),
                    )
                nc.vector.tensor_scalar_add(kT[:, ts(n, 512)], kacc, bk_sb)
            for kj in range(4):
                emit_v(kj)
            emit_q_chunk(0)

            # ------------- attention with deferred epilogue -------------
            prev_av = None
            for qi in range(16):
                # normalize chunk qi-1 BEFORE reallocating av (WAR via Tile)
                aos = None
                if qi >= 1:
                    aos = emit_normalize(qi - 1, prev_av)
                av = psum.tile([128, 512], f32, tag="av", name="av", bufs=1)
                prev_av = av
                proj_tasks = (
                    [((qi - 1) * 2 + tl, j) for tl in range(2) for j in range(2)]
                    if qi >= 1 else []
                )
                p_tiles = {}
                for m in range(35):
                    if m < 32:
                        s = psum.tile([128, 1024], f32, tag="big", name="s",
                                      bufs=3)
                        nc.tensor.matmul(
                            s[:, 0:256], kT[0:64, ts(m, 128)],
                            qT[0:64, ts(qi, 256)], start=True, stop=True,
                        )
                        nc.tensor.matmul(
                            s[:, 512:768], kT[64:128, ts(m, 128)],
                            qT[64:128, ts(qi, 256)], start=True, stop=True,
                        )
                        sv = s.rearrange("p (b c) -> p b c", b=2)[:, :, 0:256]
                        p = ppool.tile([128, 512], fp16, tag="p")
                        pv = p.rearrange("p (b c) -> p b c", b=2)
                        if m in ACT_KIS:
                            nc.scalar.activation(pv, sv, AF.Exp, scale=0.125)
                        else:
                            nc.vector.tensor_scalar(
                                pv.bitcast(u16), sv, SCH_A, SCH_B,
                                ALU.mult, ALU.add,
                            )
                        p_tiles[m] = p
                    if m >= 3:
                        kj = m - 3
                        p = p_tiles.pop(kj)
                        for h in range(2):
                            for qt in range(2):
                                g = h * 2 + qt
                                # start clears has_written bank-wide: only
                                # the first group may set it
                                nc.tensor.matmul(
                                    av[:, g * AVS:g * AVS + 65],
                                    p[:, h * 256 + qt * 128:
                                      h * 256 + (qt + 1) * 128],
                                    v_nat[:, 2 * kj + h, :],
                                    start=(kj == 0 and g == 0),
                                    stop=(kj == 31),
                                    skip_group_check=True,
                                )
                    if qi == 0 and 2 <= m <= 29:
                        # JIT V: fill chunk 0's exp-bound PE idle; V(kj)
                        # lands 5 key-blocks ahead of its AV use
                        emit_v(m + 2)
                    if aos is not None and m in (7, 17):
                        tl = (m - 7) // 10
                        emit_transpose(qi - 1, tl, aos[tl])
                    if proj_tasks and m in (9, 15, 21, 27):
                        emit_proj(*proj_tasks.pop(0),
                                  evict=("act" if m in (9, 21) else "dve"))
                    if qi < 15 and m == 34:
                        # end-of-chunk Q burst overlaps the DVE normalize
                        emit_q_chunk(qi + 1)
            # tail: epilogue of the final chunk, pipelined per tok-tile
            aos = emit_normalize(15, prev_av)
            for tl in range(2):
                emit_transpose(15, tl, aos[tl])
            for i, (tl, j) in enumerate(
                    [(tl, j) for tl in range(2) for j in range(2)]):
                emit_proj(15 * 2 + tl, j, evict=("act" if i % 2 else "dve"))

    nc.compile()
    _NC_CACHE[(reps, trace_sim)] = nc
    return nc


def make_in_maps(x, W_qkv, b_qkv, W_proj):
    x2 = np.asarray(x, dtype=np.float32).reshape(N, DIM)
    xTv = np.ascontiguousarray(x2.T.astype(np.float16))
    W_qkv = np.asarray(W_qkv, dtype=np.float32)
    W16 = W_qkv.astype(np.float16)
    b_qkv = np.asarray(b_qkv, dtype=np.float32)
    Wp16 = np.asarray(W_proj, dtype=np.float32).astype(np.float16)
    maps = []
    for m in range(NUM_CORES):
        h0 = m * DPC
        maps.append({
            "xT": xTv,
            "wq": np.ascontiguousarray(W16[:, h0:h0 + DPC]),
            "wk": np.ascontiguousarray(W16[:, DIM + h0:DIM + h0 + DPC]),
            "wv": np.ascontiguousarray(W16[:, 2 * DIM + h0:2 * DIM + h0 + DPC]),
            "wp": np.ascontiguousarray(Wp16[h0:h0 + DPC, :]),
            "bq": np.ascontiguousarray(b_qkv[h0:h0 + DPC].reshape(DPC, 1)),
            "bk": np.ascontiguousarray(
                b_qkv[DIM + h0:DIM + h0 + DPC].reshape(DPC, 1)),
            "bv": np.ascontiguousarray(
                b_qkv[2 * DIM + h0:2 * DIM + h0 + DPC]
                .astype(np.float16).reshape(1, DPC)),
            "ones": np.ones((1, 1), dtype=np.float16),
        })
    return maps


def kernel(x, W_qkv, b_qkv, W_proj, b_proj, _reps=1):
    from concourse.bass_utils import run_bass_kernel_spmd

    nc = build_nc(_reps)
    maps = make_in_maps(x, W_qkv, b_qkv, W_proj)
    res = run_bass_kernel_spmd(nc, maps, list(range(NUM_CORES)))
    total = np.zeros((N, DIM), dtype=np.float32)
    for r in res.results:
        total += r["out"].astype(np.float32)
    total = total + np.asarray(b_proj, dtype=np.float32)[None, :]
    return total.reshape(1, N, DIM).astype(np.float32)
